# revision 3
# baseline (speedup 1.0000x reference)
"""DeeperGCN (3x GENConv, softmax aggregation) Trainium2 kernel, 8 NeuronCores.

Strategy (standard distributed-GNN node partitioning, per sharding hint):
  - Nodes are sharded across 8 cores by contiguous dst ranges (6250/core,
    padded to 6272 = 49*128 slots). Within a core, nodes are bin-packed
    into 49 groups of 128 slots so each group has ~equal incident edges;
    groups are then ranked by edge count so that rank r has a uniform
    block budget W[r] across cores (SPMD: one compiled program).
  - Small weights are replicated; the per-layer gather table z (node
    features [50176, 64] in "slot" order) is replicated via AllGather.
  - Edges live on the core owning their dst. Per dst-rank, edges fill
    W[r] blocks of 128 lanes. Message build: HWDGE reads precomputed
    edge encodings ea (edge-major) while indirect DMAs gather z[src] rows
    (256B, one index per partition — the stock-runtime HW contract);
    the DVE adds them, so msg = relu(z_src + ea + b + eps). The Q7
    SWDGE desc-gen (~1.26us per 128-edge indirect DMA) is the pacing
    resource, so everything else (MLP/LN/publish of the finished node
    chunks) is interleaved between conv ranks to hide under it.
  - Softmax aggregation: exp without max-subtraction (values bounded),
    then segment sums D = sum(e), U = sum(msg*e) on the TensorEngine as
    [e|u]^T @ onehot(dst) per 128-edge block, accumulated in PSUM per
    rank. agg = U/D. Max-subtraction is skipped because s = t*msg is in
    [0, ~8] for these scales (exp safe in fp32) and softmax is
    shift-invariant, so results match the reference within fp32 rounding.
  - All node-level compute (MLP, LayerNorms) runs channel-major
    ([ch, nodes]) so per-channel affines are per-partition scalars, and
    matmuls stream 512-node chunks with layer-stationary weights.
    Per-node LN stats use ones-vector matmuls + PE row broadcasts.
"""

import numpy as np

# problem constants (hardcoded per harness contract)
N, E = 50000, 800000
DC, EC, H, L = 128, 16, 64, 3
EPS_MSG = 1e-7
LN_EPS = 1e-5
NCORES = 8

RPC = 4                         # conv ranks per interleaved MLP chunk


# ----------------------------------------------------------------------------
# Host-side graph partitioning
# ----------------------------------------------------------------------------

def _prep(x, edge_index, edge_attr, n, ncores):
    """Partition nodes/edges. Returns per-core arrays + global metadata.

    Nodes are LPT-packed into groups of 128 slots balancing incident
    edges, then groups are ranked by edge count (desc) per core and slots
    renumbered by rank so the per-rank block count W[r] (max over cores)
    is nearly tight for every core.
    """
    import heapq

    npc = n // ncores                      # owned nodes per core
    ngroups = (npc + 127) // 128           # groups of 128 slots
    npad = ngroups * 128                   # padded owned slots

    src = np.asarray(edge_index[0], dtype=np.int64)
    dst = np.asarray(edge_index[1], dtype=np.int64)

    deg = np.bincount(dst, minlength=n)
    owner = dst // npc
    np.clip(owner, 0, ncores - 1, out=owner)

    # slot assignment: per core, LPT bin-packing of nodes into groups
    lane_of = np.empty(n, dtype=np.int64)   # lane within group
    grp_of = np.empty(n, dtype=np.int64)    # group id (pre-ranking)
    grp_cnt = np.zeros((ncores, ngroups), dtype=np.int64)
    for c in range(ncores):
        lo, hi = c * npc, (c + 1) * npc if c < ncores - 1 else n
        nodes = np.arange(lo, hi)
        order = nodes[np.argsort(-deg[lo:hi], kind="stable")]
        heap = [(0, g) for g in range(ngroups)]
        heapq.heapify(heap)
        used = np.zeros(ngroups, dtype=np.int64)
        pending = []
        for nd in order:
            while True:
                cnt, g = heapq.heappop(heap)
                if used[g] < 128:
                    break
                pending.append((cnt, g))
            for it in pending:
                heapq.heappush(heap, it)
            pending.clear()
            grp_of[nd] = g
            lane_of[nd] = used[g]
            used[g] += 1
            grp_cnt[c, g] = cnt + deg[nd]
            heapq.heappush(heap, (cnt + deg[nd], g))

    # rank groups by edge count desc per core; renumber slots by rank
    rank_of_grp = np.empty((ncores, ngroups), dtype=np.int64)
    cnt_sorted = np.empty((ncores, ngroups), dtype=np.int64)
    for c in range(ncores):
        order = np.argsort(-grp_cnt[c], kind="stable")
        rank_of_grp[c, order] = np.arange(ngroups)
        cnt_sorted[c] = grp_cnt[c, order]

    node_owner = np.minimum(np.arange(n) // npc, ncores - 1)
    slot_of = rank_of_grp[node_owner, grp_of] * 128 + lane_of
    # z-table row: chunk-major, core-minor within chunk (so per-chunk
    # AllGathers concatenate into the right place). Chunks are 4 ranks
    # (512 slots); the last chunk holds the remaining 128 slots.
    CHS = 512
    chunk_of = slot_of // CHS
    s_loc = slot_of - chunk_of * CHS
    csz = np.minimum(npad - chunk_of * CHS, CHS)   # slots per chunk
    grow = chunk_of * CHS * ncores + node_owner * csz + s_loc

    # per-rank block budget, uniform across cores
    w_list = [int(np.ceil(cnt_sorted[:, r].max() / 128.0)) for r in range(ngroups)]
    w_list = [max(w, 1) for w in w_list]
    base = np.concatenate([[0], np.cumsum(w_list)])
    nblk = int(base[-1])

    # per-core edge arrays
    e_owner = owner
    e_rank = rank_of_grp[e_owner, grp_of[dst]]
    cores = []
    for c in range(ncores):
        sel = np.nonzero(e_owner == c)[0]
        r_of_e = e_rank[sel]
        order = np.argsort(r_of_e, kind="stable")
        sel = sel[order]
        r_of_e = r_of_e[order]

        gidx = np.zeros((128, nblk), dtype=np.int32)
        dstrel = np.full((128, nblk), -1.0, dtype=np.float32)
        eattrT = np.zeros((17, nblk * 128), dtype=np.float32)

        counts = np.bincount(r_of_e, minlength=ngroups)
        starts = np.concatenate([[0], np.cumsum(counts)])
        for r in range(ngroups):
            eg = sel[starts[r]:starts[r + 1]]
            ne = eg.shape[0]
            q = np.arange(ne)
            j = base[r] + q // 128
            p = q % 128
            gidx[p, j] = grow[src[eg]].astype(np.int32)
            dstrel[p, j] = lane_of[dst[eg]].astype(np.float32)
            col = j * 128 + p
            eattrT[:16, col] = np.asarray(edge_attr[eg], dtype=np.float32).T
            eattrT[16, col] = 1.0

        # x in slot order, transposed
        lo, hi = c * npc, (c + 1) * npc if c < ncores - 1 else n
        xT = np.zeros((128, npad), dtype=np.float32)
        xs = np.asarray(x[lo:hi], dtype=np.float32)
        xT[:, slot_of[lo:hi]] = xs.T
        cores.append(dict(gidx=gidx, dstrel=dstrel, eattrT=eattrT, xT=xT))

    meta = dict(npc=npc, ngroups=ngroups, npad=npad, w_list=w_list,
                nblk=nblk, slot_of=slot_of)
    return cores, meta


# ----------------------------------------------------------------------------
# Bass program
# ----------------------------------------------------------------------------

def _build(nc, tc, cfg):
    """Emit the kernel into TileContext tc."""
    import concourse.bass as bass
    import concourse.mybir as mybir
    from concourse.bass import IndirectOffsetOnAxis, broadcast_tensor_aps
    from contextlib import ExitStack

    dt = mybir.dt
    f32 = dt.float32
    Alu = mybir.AluOpType
    Act = mybir.ActivationFunctionType

    NG = cfg["ngroups"]
    WL = cfg["w_list"]
    BASE = [0]
    for w in WL:
        BASE.append(BASE[-1] + w)
    WMAX = max(WL)
    NPAD = cfg["npad"]
    NBLK = BASE[-1]
    NCO = cfg["ncores"]
    io = cfg["io"]

    CH = 512                      # node chunk for channel-major matmuls

    ctx = ExitStack()
    with ctx:
        const = ctx.enter_context(tc.tile_pool(name="const", bufs=1))
        dram = ctx.enter_context(tc.tile_pool(name="dram", bufs=1, space="DRAM"))

        # ---- resident SBUF constants ----
        nodeW = const.tile([DC, H], f32)
        edgeW = const.tile([EC + 1, H], f32)
        mlp1W = const.tile([H + 1, L * 2 * H], f32)
        mlp2W = const.tile([2 * H, L * H], f32)
        iota = const.tile([128, 128], f32)
        ident = const.tile([128, 128], f32)
        scal = const.tile([128, 32], f32)
        gidx = const.tile([128, NBLK], dt.int32)
        dstrel = const.tile([128, NBLK], f32)
        nc.sync.dma_start(nodeW[:], io["node_W"][:])
        nc.sync.dma_start(edgeW[:], io["edge_W_aug"][:])
        nc.sync.dma_start(mlp1W[:], io["mlp1_W_aug"][:])
        nc.sync.dma_start(mlp2W[:], io["mlp2_W"][:])
        nc.sync.dma_start(iota[:], io["iota"][:])
        nc.sync.dma_start(ident[:], io["ident"][:])
        nc.sync.dma_start(scal[:], io["scal"][:])
        nc.sync.dma_start(gidx[:], io["gidx"][:])
        nc.sync.dma_start(dstrel[:], io["dstrel"][:])

        ones_c = const.tile([128, 1], f32)
        ones_r = const.tile([1, 128], f32)
        nc.vector.memset(ones_c[:], 1.0)
        nc.vector.memset(ones_r[:], 1.0)

        # per-chunk node-state tiles (single-tile dep tracking would
        # serialize the whole MLP/LN pipeline through one big tile)
        NCHUNK = (NG + RPC - 1) // RPC
        CWID = [min(CH, NPAD - k * CH) for k in range(NCHUNK)]
        hTs = [const.tile([H, CH], f32, name=f"hT{k}") for k in range(NCHUNK)]
        zTs = [const.tile([H, CH], f32, name=f"zT{k}") for k in range(NCHUNK)]
        cTs = [const.tile([H + 1, CH], f32, name=f"cT{k}") for k in range(NCHUNK)]
        for k in range(NCHUNK):
            nc.vector.memset(cTs[k][H:H + 1, :], 1.0)

        # scal columns (must match host packing)
        C_T0 = 0            # t[l] at col l (replicated over partitions)
        C_MG = 3            # mlp_ln_g[l] at col 3+l
        C_MB = 6            # mlp_ln_b[l]
        C_BG = 9            # blk_ln_g[l] (rows 0..63)
        C_BB = 12           # blk_ln_b[l]
        C_B2 = 15           # mlp2_b[l] (rows 0..63)
        C_NB = 18           # node_b (rows 0..63)
        C_EPS = 19          # LN_EPS in every partition
        eps_ap = scal[0:1, C_EPS:C_EPS + 1]

        # ---- DRAM scratch ----
        z_loc = dram.tile([NPAD, H], f32)
        z_fulls = [dram.tile([NCO * NPAD, H], f32, name=f"zfull{i}",
                             tag=f"zf{i}") for i in range(2)]
        ea_tiles = [dram.tile([128, WL[r] * H], f32, name=f"eaedge{r}",
                              tag=f"ea{r}") for r in range(NG)]

        # ---- PSUM pools ----
        ps_a = ctx.enter_context(tc.tile_pool(name="ps_a", bufs=2, space="PSUM"))
        ps_b = ctx.enter_context(tc.tile_pool(name="ps_b", bufs=2, space="PSUM"))
        ps_c = ctx.enter_context(tc.tile_pool(name="ps_c", bufs=2, space="PSUM"))
        ps_d = ctx.enter_context(tc.tile_pool(name="ps_d", bufs=2, space="PSUM"))

        # ---- SBUF pools ----
        xt_pool = ctx.enter_context(tc.tile_pool(name="xt", bufs=2))
        ein_pool = ctx.enter_context(tc.tile_pool(name="eain", bufs=1))
        easb_pool = ctx.enter_context(tc.tile_pool(name="easb", bufs=1))
        tr_sb = ctx.enter_context(tc.tile_pool(name="tr_sb", bufs=3))
        row_sb = ctx.enter_context(tc.tile_pool(name="row_sb", bufs=2))
        zg_pool = ctx.enter_context(tc.tile_pool(name="zg", bufs=4))
        eat_pool = ctx.enter_context(tc.tile_pool(name="eat", bufs=3))
        eu_pool = ctx.enter_context(tc.tile_pool(name="eu", bufs=2))
        oh_pool = ctx.enter_context(tc.tile_pool(name="oh", bufs=2))
        rec_pool = ctx.enter_context(tc.tile_pool(name="rec", bufs=2))
        y_pool = ctx.enter_context(tc.tile_pool(name="ympool", bufs=2))

        def publish_tile(src_tile, dram_loc, t):
            """transpose 128 node-cols (global tile index t) -> dram rows."""
            lc = (t % RPC) * 128
            ps = ps_d.tile([128, 128], f32, tag="psd")
            nc.tensor.transpose(ps[:, 0:H], src_tile[0:H, lc:lc + 128],
                                ident[0:H, 0:H])
            sb = tr_sb.tile([128, H], f32)
            nc.scalar.copy(sb[:], ps[:, 0:H])
            nc.sync.dma_start(dram_loc[t * 128:(t + 1) * 128, :], sb[:])

        def allgather_chunk(zf, c0, w):
            nc.gpsimd.collective_compute(
                "AllGather", Alu.bypass,
                replica_groups=[list(range(NCO))],
                ins=[z_loc[c0:c0 + w, :].opt()],
                outs=[zf[c0 * NCO:(c0 + w) * NCO, :].opt()])

        def ln_relu_chunk(srcT, dstT, P, gcol, bcol, w):
            """dstT[:, :w] = relu(LN(srcT[:, :w]) * g + b), channel
            dim = partitions (P of them). gcol/bcol are scal column indices.
            allocates its own stats psum (mu@0, sq@32)."""
            s_sl = srcT[0:P, 0:w]
            st_ps = ps_b.tile([64, CH], f32, tag="psb")
            nc.tensor.matmul(st_ps[0:1, :w], ones_c[0:P, :], s_sl, start=True, stop=True)
            sq = row_sb.tile([128, CH], f32, tag="lnsq")
            nc.scalar.square(sq[0:P, :w], s_sl)
            nc.tensor.matmul(st_ps[32:33, :w], ones_c[0:P, :], sq[0:P, :w],
                             start=True, stop=True)
            mean = row_sb.tile([1, CH], f32, tag="lnmean")
            nc.scalar.mul(mean[:, :w], st_ps[0:1, :w], 1.0 / P)
            msq = row_sb.tile([1, CH], f32, tag="lnmsq")
            nc.scalar.square(msq[:, :w], mean[:, :w])
            nc.vector.scalar_tensor_tensor(msq[:, :w], st_ps[32:33, :w], 1.0 / P,
                                           msq[:, :w], Alu.mult, Alu.subtract)
            rstd = row_sb.tile([1, CH], f32, tag="lnrstd")
            nc.scalar.activation(rstd[:, :w], msq[:, :w], Act.Sqrt, bias=eps_ap)
            nc.vector.reciprocal(rstd[:, :w], rstd[:, :w])
            # broadcast mean/rstd across partitions via PE outer product
            mb_ps = ps_c.tile([128, CH], f32, tag="psc")
            nc.tensor.matmul(mb_ps[0:P, :w], ones_r[:, 0:P], mean[:, :w],
                             start=True, stop=True)
            rb_ps = ps_c.tile([128, CH], f32, tag="psc")
            nc.tensor.matmul(rb_ps[0:P, :w], ones_r[:, 0:P], rstd[:, :w],
                             start=True, stop=True)
            tmp = row_sb.tile([128, CH], f32, tag="lntmp")
            nc.vector.tensor_sub(tmp[0:P, :w], s_sl, mb_ps[0:P, :w])
            nc.vector.tensor_mul(tmp[0:P, :w], tmp[0:P, :w], rb_ps[0:P, :w])
            nc.scalar.activation(dstT[0:P, 0:w], tmp[0:P, :w], Act.Relu,
                                 bias=scal[0:P, bcol:bcol + 1],
                                 scale=scal[0:P, gcol:gcol + 1])

        def mlp_s0(l, k, w):
            """mlp1 matmul + copy out of PSUM."""
            ps1 = ps_a.tile([128, CH], f32, tag="psa")
            nc.tensor.matmul(ps1[:, :w], mlp1W[:, l * 2 * H:(l + 1) * 2 * H],
                             cTs[k][:, 0:w], start=True, stop=True)
            y1 = y_pool.tile([128, CH], f32, tag="y1")
            nc.scalar.copy(y1[:, :w], ps1[:, :w])
            return y1

        def mlp_s1(l, k, w, y1):
            """LN over 2H=128 channels (partitions) + relu, g/b per-partition."""
            st_ps = ps_b.tile([64, CH], f32, tag="psb")
            nc.tensor.matmul(st_ps[0:1, :w], ones_c[:], y1[:, :w],
                             start=True, stop=True)
            sq = y_pool.tile([128, CH], f32, tag="lnsq")
            nc.scalar.square(sq[:, :w], y1[:, :w])
            nc.tensor.matmul(st_ps[32:33, :w], ones_c[:], sq[:, :w],
                             start=True, stop=True)
            mean = row_sb.tile([1, CH], f32, tag="lnmean")
            nc.scalar.mul(mean[:, :w], st_ps[0:1, :w], 1.0 / 128.0)
            msq = row_sb.tile([1, CH], f32, tag="lnmsq")
            nc.scalar.square(msq[:, :w], mean[:, :w])
            nc.vector.scalar_tensor_tensor(msq[:, :w], st_ps[32:33, :w], 1.0 / 128.0,
                                           msq[:, :w], Alu.mult, Alu.subtract)
            rstd = row_sb.tile([1, CH], f32, tag="lnrstd")
            nc.scalar.activation(rstd[:, :w], msq[:, :w], Act.Sqrt, bias=eps_ap)
            nc.vector.reciprocal(rstd[:, :w], rstd[:, :w])
            mb_ps = ps_c.tile([128, CH], f32, tag="psc")
            nc.tensor.matmul(mb_ps[:, :w], ones_r[:], mean[:, :w],
                             start=True, stop=True)
            rb_ps = ps_c.tile([128, CH], f32, tag="psc")
            nc.tensor.matmul(rb_ps[:, :w], ones_r[:], rstd[:, :w],
                             start=True, stop=True)
            y1n = y_pool.tile([128, CH], f32, tag="y1n")
            nc.vector.tensor_sub(y1n[:, :w], y1[:, :w], mb_ps[:, :w])
            nc.vector.tensor_mul(y1n[:, :w], y1n[:, :w], rb_ps[:, :w])
            nc.scalar.activation(y1n[:, :w], y1n[:, :w], Act.Relu,
                                 bias=scal[:, C_MB + l:C_MB + l + 1],
                                 scale=scal[:, C_MG + l:C_MG + l + 1])
            return y1n

        def mlp_s2(l, k, w, y1n):
            """mlp2 matmul + h update."""
            ps2 = ps_a.tile([128, CH], f32, tag="psa")
            nc.tensor.matmul(ps2[0:H, :w], mlp2W[:, l * H:(l + 1) * H],
                             y1n[:, :w], start=True, stop=True)
            if l == 0:
                nc.vector.tensor_scalar_add(hTs[k][:, 0:w], ps2[0:H, :w],
                                            scal[0:H, C_B2:C_B2 + 1])
            else:
                nc.vector.scalar_tensor_tensor(
                    hTs[k][:, 0:w], ps2[0:H, :w],
                    scal[0:H, C_B2 + l:C_B2 + l + 1], hTs[k][:, 0:w],
                    Alu.add, Alu.add)

        def conv_rank(l, r):
            """messages + softmax aggregation for dst-rank r of layer l."""
            W = WL[r]
            B = BASE[r]
            if l == 0:
                # fused one-time edge encoding: ea = eattrT.T @ edge_W_aug,
                # consumed directly from SBUF and stashed to DRAM for l>0
                ein = ein_pool.tile([EC + 1, WMAX * 128], f32)
                nc.sync.dma_start(ein[:, :W * 128],
                                  io["eattrT"][:, B * 128:(B + W) * 128])
                eat = easb_pool.tile([128, WMAX * H], f32)
                for j0 in range(0, W, 8):
                    jn = min(8, W - j0)
                    ps = ps_a.tile([128, 512], f32, tag="psa")
                    for j in range(j0, j0 + jn):
                        nc.tensor.matmul(
                            ps[:, (j - j0) * H:(j - j0 + 1) * H],
                            ein[:, j * 128:(j + 1) * 128], edgeW[:],
                            start=True, stop=True)
                    nc.scalar.copy(eat[:, j0 * H:(j0 + jn) * H], ps[:, :jn * H])
                nc.sync.dma_start(ea_tiles[r][:], eat[:, :W * H])
            else:
                eat = eat_pool.tile([128, WMAX * H], f32)
                nc.sync.dma_start(eat[:, :W * H], ea_tiles[r][:])
            zg = zg_pool.tile([128, WMAX * H], f32)
            zg3 = zg[:].rearrange("p (s c) -> p s c", c=H)
            # HW contract: one index per partition per indirect DMA.
            zf = z_fulls[l % 2]
            for j in range(W):
                nc.gpsimd.indirect_dma_start(
                    out=zg3[:, j, :], out_offset=None,
                    in_=zf[:],
                    in_offset=IndirectOffsetOnAxis(
                        ap=gidx[:, B + j:B + j + 1], axis=0))
            nc.vector.tensor_add(zg[:, :W * H], zg[:, :W * H], eat[:, :W * H])
            # msg = relu(z_src + ea + b + eps)
            nc.scalar.activation(zg[:, :W * H], zg[:, :W * H], Act.Relu)
            eu = eu_pool.tile([128, WMAX, 2 * H], f32)
            nc.scalar.activation(eu[:, :W, 0:H], zg3[:, :W, :], Act.Exp,
                                 scale=scal[:, C_T0 + l:C_T0 + l + 1])
            nc.vector.tensor_mul(eu[:, :W, H:2 * H], zg3[:, :W, :],
                                 eu[:, :W, 0:H])
            oh = oh_pool.tile([128, WMAX, 128], f32)
            i_ap, d_ap = broadcast_tensor_aps(
                iota[:].rearrange("p (o f) -> p o f", o=1),
                dstrel[:, B:B + W].rearrange("p (s o) -> p s o", o=1))
            nc.vector.tensor_tensor(oh[:, :W, :], i_ap, d_ap, op=Alu.is_equal)
            ps = ps_d.tile([128, 128], f32, tag="psd")
            for j in range(W):
                nc.tensor.matmul(ps[:], eu[:, j, :], oh[:, j, :],
                                 start=(j == 0), stop=(j == W - 1))
            rec = rec_pool.tile([H, 128], f32)
            nc.vector.reciprocal(rec[:], ps[0:H, :])
            # agg + GENConv root residual (conv-input z)
            kc, lc = r // RPC, (r % RPC) * 128
            cols = slice(lc, lc + 128)
            nc.vector.tensor_mul(cTs[kc][0:H, cols], ps[H:2 * H, :], rec[:])
            nc.vector.tensor_add(cTs[kc][0:H, cols], cTs[kc][0:H, cols],
                                 zTs[kc][0:H, cols])

        # ---- encoder: hT = node_W.T @ xT + node_b; z0 = hT; publish ----
        # (staged: xt prefetch one chunk ahead of compute/publish)
        def enc_load(k):
            xt = xt_pool.tile([DC, CH], f32)
            nc.sync.dma_start(xt[:, :CWID[k]], io["xT"][:, k * CH:k * CH + CWID[k]])
            return xt

        def enc_body(k, xt):
            w = CWID[k]
            ps = ps_a.tile([128, CH], f32, tag="psa")
            nc.tensor.matmul(ps[0:H, :w], nodeW[:], xt[:, :w], start=True, stop=True)
            nc.vector.tensor_scalar_add(hTs[k][:, 0:w], ps[0:H, :w],
                                        scal[0:H, C_NB:C_NB + 1])
            nc.vector.tensor_copy(zTs[k][:, 0:w], hTs[k][:, 0:w])
            for t in range(k * RPC, k * RPC + w // 128):
                publish_tile(hTs[k], z_loc, t)
            allgather_chunk(z_fulls[0][:], k * CH, w)

        enc_xt = {}
        for i in range(NCHUNK + 1):
            if i < NCHUNK:
                enc_xt[i] = enc_load(i)
            if i >= 1:
                enc_body(i - 1, enc_xt.pop(i - 1))

        # ---- layers: gather-paced conv phase, then software-pipelined
        # MLP/LN/publish phase (stage s of chunk k emits alongside stage
        # s+1 of chunk k-1 so each engine queue stays dependency-free) ----
        for l in range(L):
            for r in range(NG):
                conv_rank(l, r)

            def tail_s3(k):
                w = CWID[k]
                if l < L - 1:
                    ln_relu_chunk(hTs[k], zTs[k], H, C_BG + l + 1,
                                  C_BB + l + 1, w)
                else:
                    ln_relu_chunk(hTs[k], zTs[k], H, C_BG, C_BB, w)

            def tail_s4(k):
                w = CWID[k]
                r0 = k * RPC
                if l < L - 1:
                    for t in range(r0, r0 + w // 128):
                        publish_tile(zTs[k], z_loc, t)
                    allgather_chunk(z_fulls[(l + 1) % 2][:], k * CH, w)
                else:
                    for t in range(r0, r0 + w // 128):
                        lc = (t % RPC) * 128
                        ps = ps_d.tile([128, 128], f32, tag="psd")
                        nc.tensor.transpose(ps[:, 0:H],
                                            zTs[k][0:H, lc:lc + 128],
                                            ident[0:H, 0:H])
                        sb = tr_sb.tile([128, H], f32)
                        nc.scalar.copy(sb[:], ps[:, 0:H])
                        nc.sync.dma_start(io["out"][t * 128:(t + 1) * 128, :],
                                          sb[:])

            st_y1, st_y1n = {}, {}
            for i in range(NCHUNK + 4):
                if i < NCHUNK:
                    st_y1[i] = mlp_s0(l, i, CWID[i])
                if 1 <= i < NCHUNK + 1:
                    k = i - 1
                    st_y1n[k] = mlp_s1(l, k, CWID[k], st_y1.pop(k))
                if 2 <= i < NCHUNK + 2:
                    k = i - 2
                    mlp_s2(l, k, CWID[k], st_y1n.pop(k))
                if 3 <= i < NCHUNK + 3:
                    tail_s3(i - 3)
                if 4 <= i < NCHUNK + 4:
                    tail_s4(i - 4)


# ----------------------------------------------------------------------------
# Weight packing (host)
# ----------------------------------------------------------------------------

def _pack_weights(inp):
    f = np.float32
    node_W = np.asarray(inp["node_W"], f)                    # [128, 64]
    edge_W_aug = np.concatenate(
        [np.asarray(inp["edge_W"], f),
         (np.asarray(inp["edge_b"], f) + EPS_MSG)[None, :]], axis=0)  # [17,64]
    m1 = np.asarray(inp["mlp1_W"], f)                        # [L, 64, 128]
    m1b = np.asarray(inp["mlp1_b"], f)                       # [L, 128]
    mlp1_W_aug = np.zeros((H + 1, L * 2 * H), f)
    for l in range(L):
        mlp1_W_aug[:H, l * 2 * H:(l + 1) * 2 * H] = m1[l]
        mlp1_W_aug[H, l * 2 * H:(l + 1) * 2 * H] = m1b[l]
    m2 = np.asarray(inp["mlp2_W"], f)                        # [L, 128, 64]
    mlp2_W = np.concatenate([m2[l] for l in range(L)], axis=1)  # [128, L*64]
    iota = np.tile(np.arange(128, dtype=f)[None, :], (128, 1))
    ident = np.eye(128, dtype=f)
    scal = np.zeros((128, 32), f)
    t = np.asarray(inp["t"], f)
    for l in range(L):
        scal[:, 0 + l] = t[l]
        scal[:, 3 + l] = np.asarray(inp["mlp_ln_g"], f)[l]
        scal[:, 6 + l] = np.asarray(inp["mlp_ln_b"], f)[l]
        scal[:H, 9 + l] = np.asarray(inp["blk_ln_g"], f)[l]
        scal[:H, 12 + l] = np.asarray(inp["blk_ln_b"], f)[l]
        scal[:H, 15 + l] = np.asarray(inp["mlp2_b"], f)[l]
    scal[:H, 18] = np.asarray(inp["node_b"], f)
    scal[:, 19] = LN_EPS
    return dict(node_W=node_W, edge_W_aug=edge_W_aug, mlp1_W_aug=mlp1_W_aug,
                mlp2_W=mlp2_W, iota=iota, ident=ident, scal=scal)


# ----------------------------------------------------------------------------
# Driver
# ----------------------------------------------------------------------------

def _make_program(inputs, n, ncores):
    import concourse.bacc as bacc
    import concourse.tile as tile
    import concourse.mybir as mybir

    x = np.asarray(inputs["x"], np.float32)
    edge_index = np.asarray(inputs["edge_index"])
    edge_attr = np.asarray(inputs["edge_attr"], np.float32)

    cores, meta = _prep(x, edge_index, edge_attr, n, ncores)
    weights = _pack_weights(inputs)

    NG, WLIST, NPAD, NBLK = (meta["ngroups"], meta["w_list"], meta["npad"],
                             meta["nblk"])

    nc = bacc.Bacc("TRN2", target_bir_lowering=False, debug=False,
                   enable_asserts=False, num_devices=ncores)
    dt = mybir.dt
    f32 = dt.float32

    io = {}
    io["xT"] = nc.dram_tensor("xT", [DC, NPAD], f32, kind="ExternalInput").ap()
    io["eattrT"] = nc.dram_tensor("eattrT", [EC + 1, NBLK * 128], f32,
                                  kind="ExternalInput").ap()
    io["gidx"] = nc.dram_tensor("gidx", [128, NBLK], dt.int32,
                                kind="ExternalInput").ap()
    io["dstrel"] = nc.dram_tensor("dstrel", [128, NBLK], f32,
                                  kind="ExternalInput").ap()
    for k, v in weights.items():
        io[k] = nc.dram_tensor(k, list(v.shape), f32, kind="ExternalInput").ap()
    io["out"] = nc.dram_tensor("out", [NPAD, H], f32, kind="ExternalOutput").ap()

    cfg = dict(ngroups=NG, w_list=WLIST, npad=NPAD, ncores=ncores, io=io)
    with tile.TileContext(nc) as tc:
        _build(nc, tc, cfg)
    nc.compile()

    in_maps = []
    for c in range(ncores):
        m = dict(xT=cores[c]["xT"], eattrT=cores[c]["eattrT"],
                 gidx=cores[c]["gidx"], dstrel=cores[c]["dstrel"])
        m.update(weights)
        in_maps.append(m)
    return nc, in_maps, meta


def _unshard(results, meta, n, ncores):
    npc, slot_of = meta["npc"], meta["slot_of"]
    out = np.empty((n, H), np.float32)
    for c in range(ncores):
        lo, hi = c * npc, (c + 1) * npc if c < ncores - 1 else n
        out[lo:hi] = results[c]["out"][slot_of[lo:hi]]
    return out


def _run(inputs, n, ncores, trace=False):
    import concourse.bass_utils as bass_utils
    nc, in_maps, meta = _make_program(inputs, n, ncores)
    res = bass_utils.run_bass_kernel_spmd(
        nc, in_maps, core_ids=list(range(ncores)), trace=trace)

    return _unshard(res.results, meta, n, ncores), res


def kernel(**inputs):
    out, _ = _run(inputs, N, NCORES)
    return out
